# revision 36
# baseline (speedup 1.0000x reference)
"""MoE (Gemma-style 8-expert top-2) Trainium2 kernel — intermediate-sharded.

Strategy (tensor-parallel over the intermediate dim, 8 NeuronCores):
  - Host: merge duplicate (token, expert) assignments, build per-expert token
    lists, gather+transpose x into a packed xT stream (one contiguous block
    per expert, zero-padded to a multiple of 4).  Weights are fp16 and
    prepacked per core: core j owns columns [j*512, (j+1)*512) of the
    intermediate dim for ALL 8 experts, so every core executes the exact
    same (perfectly balanced) sequence of matmuls — sum_e 192*C_e cycles —
    instead of being held hostage by the most-loaded expert.
  - Device (core j), for each expert e with C_e tokens, all transposed
    layout so matmuls use natural weight layouts:
        gateT[i, c] = sum_h Wg[h, j*512+i] * xT[h, c]   (i in [0,512))
        upT   likewise
        hT    = gelu_tanh(gateT) * upT        [512, C]  fp16 in SBUF
        yT[h, c] = sum_i Wd[j*512+i, h] * hT[i, c]      [2048, C] partial!
    Emission is software-pipelined: the down-projection of expert e-1 is
    woven between the gate/up tiles of expert e so the PE never waits for
    the gelu/mul (DVE/ACT) results of the expert it just finished.
  - Host: combine — sum the 8 per-core partial yT (fp32), then
    out[t] += route[t,e] * yT_e[:, pos].T with route matching the
    reference's scatter-add exactly.
"""

import numpy as np

import concourse.bass as bass
import concourse.mybir as mybir
import concourse.tile as tile
from concourse import bacc


def _install_ntff_hook_shim():
    """The agent image's `antenv` lacks `axon_hooks`, which bass_utils
    imports unconditionally when tracing under axon.  Provide the module
    and register the ctypes-based NTFF profile hook so BASS_TRACE=1 yields
    real HW profiles.  Degrades silently if anything is missing."""
    import sys
    import types

    try:
        import antenv

        try:
            from antenv import axon_hooks  # noqa: F401

            return
        except ImportError:
            pass
        mod = types.ModuleType("antenv.axon_hooks")
        mod._hook = None
        mod.set_axon_ntff_profile_hook = lambda h: setattr(mod, "_hook", h)
        mod.get_axon_ntff_profile_hook = lambda: mod._hook
        sys.modules["antenv.axon_hooks"] = mod
        antenv.axon_hooks = mod
        import os

        so_path = "/opt/axon/libaxon_pjrt.so"
        if os.path.exists(so_path):
            from trn_agent_boot.trn_boot import _ntff_profile_via_ctypes

            mod._hook = _ntff_profile_via_ctypes(so_path)
    except Exception:
        pass


_install_ntff_hook_shim()

from concourse.bass_utils import run_bass_kernel_spmd

H = 2048
I = 4096
E = 8
P = 128
NCORES = 8
ISL = I // NCORES  # 512-wide intermediate slice per core
MI = ISL // P  # 4 gate/up output tiles per expert
KH = H // P  # 16 contraction chunks for gate/up
KI = ISL // P  # 4 contraction chunks for down
MH = H // P  # 16 down output tiles
CMAX = 504  # max token-columns per work item (PSUM bank = 512 fp32)
CPAD = 2
WBLK = MI * KH * P  # 8192 weight cols per expert (gate/up); same for down
F32 = mybir.dt.float32
F16 = mybir.dt.float16

# Results of the last device run (for test harnesses to inspect profiling).
LAST_RESULTS = None

_PROGRAM_CACHE: dict[tuple, "bass.Bass"] = {}


def _build_program(items: tuple[tuple[int, int], ...]) -> "bass.Bass":
    """Bass program for one core: for each (expert, C) item, the full expert
    MLP on its I-slice.  Identical across cores (weights differ)."""
    n = len(items)
    xcols = sum(KH * c for _, c in items)
    ycols = sum(MH * c for _, c in items)

    nc = bacc.Bacc("TRN2", target_bir_lowering=False)

    XT = nc.dram_tensor("XT", [P, xcols], F16, kind="ExternalInput")
    WG = nc.dram_tensor("WG", [P, E * WBLK], F16, kind="ExternalInput")
    WU = nc.dram_tensor("WU", [P, E * WBLK], F16, kind="ExternalInput")
    WD = nc.dram_tensor("WD", [P, E * WBLK], F16, kind="ExternalInput")
    Y = nc.dram_tensor("Y", [P, ycols], F16, kind="ExternalOutput")

    XT_a, WG_a, WU_a, WD_a, Y_a = XT.ap(), WG.ap(), WU.ap(), WD.ap(), Y.ap()

    gelu = mybir.ActivationFunctionType.Gelu_apprx_tanh

    xoffs, yoffs = [], []
    xo = yo = 0
    for _, c in items:
        xoffs.append(xo)
        yoffs.append(yo)
        xo += KH * c
        yo += MH * c

    with tile.TileContext(nc) as tc:
        with (
            tc.tile_pool(name="xpool", bufs=3) as xpool,
            tc.tile_pool(name="dpool", bufs=2) as dpool,
            tc.tile_pool(name="wpool", bufs=2) as wpool,
            tc.tile_pool(name="gpool", bufs=3) as gpool,
            tc.tile_pool(name="warm", bufs=1) as warm_pool,
            tc.tile_pool(name="psum", bufs=2, space="PSUM") as psum_pool,
            tc.tile_pool(name="psumw", bufs=1, space="PSUM") as psum_warm,
        ):
            hpool = ypool = dpool
            wgpool = wupool = wdpool = wpool
            psum_gu = psum_d_pool = psum_pool
            # --- PE warm-up: dummy matmuls trip the HAM clock-gate to 8/8
            # while the first DMAs land (~3.4us budget).
            wz = warm_pool.tile([P, P], F16)
            xz = warm_pool.tile([P, P], F16)
            nc.vector.memset(wz, 0.0)
            nc.vector.memset(xz, 0.0)
            psum_w = psum_warm.tile([P, P], F32, tag="warm")

            def warm_mm(count):
                for _ in range(count):
                    nc.tensor.matmul(psum_w, wz, xz, start=True, stop=True)

            # ~3.4us of continuous cold-rate PE busy trips HAM to full
            # clock; the ramp fillers below keep it there after the burst.
            # Sized past the threshold so a slow first-DMA run cannot let
            # the gate re-throttle before real matmuls take over.
            warm_mm(56)

            tiles = [None] * n  # per item: (xsb, wg_v, wu_v, wd_v)
            ysbs = [None] * n

            def emit_loads(i, first):
                e, C = items[i]
                xsb = xpool.tile([P, KH, C], F16, tag="x", name=f"x_{i}")
                xflat = xsb.rearrange("p k c -> p (k c)")
                xa = XT_a[:, xoffs[i] : xoffs[i] + KH * C]
                # gate/up weights in half-tiles (m0-m1 / m2-m3): the first
                # half's slot frees mid-item, so the next-next item's load
                # is not WAR-blocked until the whole item finishes
                HB = WBLK // 2
                wg_a = wgpool.tile([P, HB], F16, tag="wga", name=f"wga_{i}")
                wg_b = wgpool.tile([P, HB], F16, tag="wgb", name=f"wgb_{i}")
                wu_a = wupool.tile([P, HB], F16, tag="wua", name=f"wua_{i}")
                wu_b = wupool.tile([P, HB], F16, tag="wub", name=f"wub_{i}")
                wd_t = wdpool.tile([P, WBLK], F16, tag="wd", name=f"wd_{i}")
                ga = WG_a[:, e * WBLK : (e + 1) * WBLK]
                ua = WU_a[:, e * WBLK : (e + 1) * WBLK]
                da = WD_a[:, e * WBLK : (e + 1) * WBLK]

                def dwh(wt, sa, half, a, b):
                    # cols [a,b) within half-tile `half` of wg/wu
                    nc.sync.dma_start(
                        out=wt[:, a:b], in_=sa[:, half * HB + a : half * HB + b]
                    )

                def dx(a, b):  # x chunks [a,b) of KH
                    nc.sync.dma_start(
                        out=xflat[:, a * C : b * C], in_=xa[:, a * C : b * C]
                    )

                KP = KH * P  # 2048 cols per gate/up m-tile
                if first:
                    # single HWDGE ring drains FIFO: supply chunks in the
                    # order the first matmuls consume them.  Each dma_start
                    # costs ~0.7us of issue time on the sync engine, so the
                    # schedule uses few, moderately-sized transfers.
                    dwh(wg_a, ga, 0, 0, 8 * P)
                    dx(0, 2)
                    dx(2, 5)
                    dwh(wg_a, ga, 0, 8 * P, KP)
                    dx(5, 9)
                    dx(9, 16)
                    dwh(wu_a, ua, 0, 0, KP)
                    dwh(wg_a, ga, 0, KP, 2 * KP)
                    dwh(wu_a, ua, 0, KP, 2 * KP)
                    dwh(wg_b, ga, 1, 0, KP)
                    dwh(wu_b, ua, 1, 0, KP)
                    dwh(wg_b, ga, 1, KP, 2 * KP)
                    dwh(wu_b, ua, 1, KP, 2 * KP)
                else:
                    dx(0, 8)
                    dwh(wg_a, ga, 0, 0, KP)
                    dx(8, 16)
                    dwh(wg_a, ga, 0, KP, 2 * KP)
                    dwh(wu_a, ua, 0, 0, 2 * KP)
                    dwh(wg_b, ga, 1, 0, 2 * KP)
                    dwh(wu_b, ua, 1, 0, 2 * KP)
                tiles[i] = (
                    xsb,
                    (
                        wg_a.rearrange("p (m k i) -> p m k i", m=2, k=KH),
                        wg_b.rearrange("p (m k i) -> p m k i", m=2, k=KH),
                    ),
                    (
                        wu_a.rearrange("p (m k i) -> p m k i", m=2, k=KH),
                        wu_b.rearrange("p (m k i) -> p m k i", m=2, k=KH),
                    ),
                    wd_t.rearrange("p (m k i) -> p m k i", m=MH, k=KI),
                )
                wd_srcs[i] = (wd_t, da)

            def emit_wd_load(i):
                # scalar-engine ring (shared with y writebacks): keeps the
                # down-projection weights off the latency-critical sync ring.
                # The first two have no WAR deps, so without a manual wait
                # the scheduler hoists them to t=0 where their 4MB starves
                # the ramp-critical x/wg loads (measured: +8us).
                wd_t, da = wd_srcs[i]
                if i < 2:
                    with tc.tile_wait_until(0.012 + 0.010 * i):
                        nc.scalar.dma_start(out=wd_t, in_=da)
                else:
                    nc.scalar.dma_start(out=wd_t, in_=da)

            def emit_down_chunk(i, m2_range):
                """Down-projection tiles m2_range of item i + drains."""
                e, C = items[i]
                wd_v = tiles[i][3]
                hsb = hsbs[i]
                if ysbs[i] is None:
                    ysbs[i] = ypool.tile([P, MH, C], F16, tag="y", name=f"y_{i}")
                ysb = ysbs[i]
                for m2 in m2_range:
                    psum_d = psum_d_pool.tile([P, C], F32, tag="d")
                    for k2 in range(KI):
                        nc.tensor.matmul(
                            psum_d,
                            wd_v[:, m2, k2, :],
                            hsb[:, k2, :],
                            start=(k2 == 0),
                            stop=(k2 == KI - 1),
                        )
                    nc.vector.tensor_copy(ysb[:, m2, :], psum_d)

            def emit_y_dma(i, lo, hi):
                e, C = items[i]
                yflat = ysbs[i].rearrange("p m c -> p (m c)")
                # scalar-engine HWDGE queue: keeps writebacks off the sync
                # engine, whose queue carries the latency-critical loads
                nc.scalar.dma_start(
                    out=Y_a[:, yoffs[i] + lo * C : yoffs[i] + hi * C],
                    in_=yflat[:, lo * C : hi * C],
                )

            hsbs = [None] * n
            wd_srcs = [None] * n
            down_sched = [range(0, 4), range(4, 8), range(8, 12), range(12, 16)]

            emit_loads(0, True)

            for i in range(n):
                e, C = items[i]
                if i + 1 < n:
                    emit_loads(i + 1, False)
                if i >= 1:
                    emit_wd_load(i)
                xsb, wg_v, wu_v, _ = tiles[i]
                hsb = hpool.tile([P, KI, C], F16, tag="h", name=f"h_{i}")
                hsbs[i] = hsb
                if i == 0:
                    emit_wd_load(0)
                for m in range(MI):
                    wg_h = wg_v[m // 2]
                    wu_h = wu_v[m // 2]
                    mh = m % 2
                    psum_g = psum_gu.tile([P, C], F32, tag="g")
                    psum_u = psum_gu.tile([P, C], F32, tag="u")
                    for k in range(KH):
                        nc.tensor.matmul(
                            psum_g,
                            wg_h[:, mh, k, :],
                            xsb[:, k, :],
                            start=(k == 0),
                            stop=(k == KH - 1),
                        )
                        if i == 0 and m == 0:
                            # DMA-paced ramp: keep the PE busy between the
                            # first real matmuls so HAM stays at full clock
                            warm_mm(2)
                    for k in range(KH):
                        nc.tensor.matmul(
                            psum_u,
                            wu_h[:, mh, k, :],
                            xsb[:, k, :],
                            start=(k == 0),
                            stop=(k == KH - 1),
                        )
                    tg = gpool.tile([P, C], F32, tag="gelu", name=f"g_{i}_{m}")
                    nc.scalar.activation(tg, psum_g, gelu)
                    nc.vector.tensor_mul(hsb[:, m, :], tg, psum_u)
                    if i > 0:
                        emit_down_chunk(i - 1, down_sched[m])
                        if m == 1:
                            emit_y_dma(i - 1, 0, 8)
                        elif m == 3:
                            emit_y_dma(i - 1, 8, 16)
                # release previous item's tiles happens via pool cycling

            # tail: down-projection of the last item; final writeback split
            # fine so only a tiny DMA trails the last matmul
            for q in range(4):
                emit_down_chunk(n - 1, down_sched[q])
                if q < 3:
                    emit_y_dma(n - 1, 4 * q, 4 * q + 4)
            emit_y_dma(n - 1, 12, 15)
            emit_y_dma(n - 1, 15, 16)

    nc.compile()
    return nc


def _get_program(items) -> "bass.Bass":
    key = tuple(items)
    if key not in _PROGRAM_CACHE:
        _PROGRAM_CACHE[key] = _build_program(key)
    return _PROGRAM_CACHE[key]


def _pack_wgu(w16):
    """[E, H, I] fp16 -> [NCORES, P, E*WBLK]: per core j, expert-block layout
    cols = ((e*MI + m)*KH + k)*P + i  with value Wg[e][k*P+p, j*ISL+m*P+i]."""
    a = w16.reshape(E, KH, P, NCORES, MI, P).transpose(3, 2, 0, 4, 1, 5)
    return np.ascontiguousarray(a).reshape(NCORES, P, E * WBLK)


def _pack_wd(w16):
    """[E, I, H] fp16 -> [NCORES, P, E*WBLK]: cols ((e*MH+m2)*KI+k2)*P + hh
    with value Wd[e][j*ISL + k2*P + p, m2*P + hh]."""
    a = w16.reshape(E, NCORES, KI, P, MH, P).transpose(1, 3, 0, 4, 2, 5)
    return np.ascontiguousarray(a).reshape(NCORES, P, E * WBLK)


def kernel(x, selected_experts, routing_weights, Wg, Wu, Wd):
    global LAST_RESULTS
    x = np.asarray(x, dtype=np.float32)
    se = np.asarray(selected_experts).astype(np.int64)
    rw = np.asarray(routing_weights).astype(np.float32)
    Wg = np.asarray(Wg, dtype=np.float32)
    Wu = np.asarray(Wu, dtype=np.float32)
    Wd = np.asarray(Wd, dtype=np.float32)

    T, K = se.shape
    assert x.shape == (T, H) and Wg.shape == (E, H, I) and Wd.shape == (E, I, H)

    # Dense route matrix, identical to the reference's scatter-add (merges
    # duplicate expert picks within a token by summing their weights).
    flat_t = np.repeat(np.arange(T), K)
    flat_e = se.ravel()
    route = np.zeros((T, E), np.float32)
    np.add.at(route, (flat_t, flat_e), rw.ravel())
    present = np.zeros((T, E), bool)
    present[flat_t, flat_e] = True

    # Work items: (expert, token-index chunk), chunks capped at CMAX columns,
    # padded to a multiple of CPAD.  Largest first (shortest tail last).
    work = []
    for e in range(E):
        ix = np.nonzero(present[:, e])[0]
        for s in range(0, len(ix), CMAX):
            chunk = ix[s : s + CMAX]
            cpad = max(CPAD, -(-len(chunk) // CPAD) * CPAD)
            work.append((e, chunk, cpad))
    work.sort(key=lambda w: -w[2])

    items = tuple((e, c) for e, _, c in work)
    nc = _get_program(items)

    xoffs, yoffs = [], []
    xo = yo = 0
    for _, c in items:
        xoffs.append(xo)
        yoffs.append(yo)
        xo += KH * c
        yo += MH * c

    # --- pack inputs
    XT = np.zeros((P, xo), np.float16)
    for (e, ix, c), xof in zip(work, xoffs):
        blk = np.zeros((P, KH, c), np.float16)
        if len(ix):
            # x[ix].T: [H, Ca] -> [KH, P, Ca] -> [P, KH, Ca]
            blk[:, :, : len(ix)] = (
                x[ix].T.astype(np.float16).reshape(KH, P, len(ix)).transpose(1, 0, 2)
            )
        XT[:, xof : xof + KH * c] = blk.reshape(P, KH * c)

    WGp = _pack_wgu(Wg.astype(np.float16))
    WUp = _pack_wgu(Wu.astype(np.float16))
    WDp = _pack_wd(Wd.astype(np.float16))

    in_maps = [
        {"XT": XT, "WG": WGp[j], "WU": WUp[j], "WD": WDp[j]} for j in range(NCORES)
    ]
    res = run_bass_kernel_spmd(nc, in_maps, core_ids=list(range(NCORES)))
    LAST_RESULTS = res

    # --- combine: sum partial yT across cores, then weighted scatter-add
    Ysum = np.zeros((P, yo), np.float32)
    for j in range(NCORES):
        Ysum += res.results[j]["Y"].astype(np.float32)

    out = np.zeros((T, H), np.float32)
    for (e, ix, c), yof in zip(work, yoffs):
        if len(ix) == 0:
            continue
        blk = Ysum[:, yof : yof + MH * c].reshape(P, MH, c)[:, :, : len(ix)]
        # [p, m2, c] -> [c, m2, p] -> [c, H]
        y_e = np.ascontiguousarray(blk.transpose(2, 1, 0)).reshape(len(ix), H)
        out[ix] += route[ix, e][:, None] * y_e
    return out
